# revision 31
# baseline (speedup 1.0000x reference)
"""Trainium2 Bass kernel for LocalSemanticAlignment (sparse_attention).

Pipeline (reference semantics):
  masks   = parse[:,1:] downsampled 256->64 (nearest, stride-4)
  ufb     = bilinear-AC downsample of unalign_fb to 64x64        (host)
  fan/fbn = per-channel-centered, per-column L2-normalized fa/fb (host)
  S[q,p]  = fbn^T fan                                            (device, fp16 matmul)
  per class k: w_k = where(mask_b[q], exp(alpha*S - C), exp(-C)) masked
  softmax over q (shift C is exact; see CSHIFT)
  warped_k = ufb @ softmax  ->  combined over k with mask_a / counts
  output  = bilinear-AC upsample of aligned to 256x256           (host)

Key identity used on device: w_k = mask_b[q]*exp(aS-C) + (1-mask_b[q])e^-C, so
  numer_k = (ufb*mask_b[k])^T @ E + const_k,  denom_k = mask_b[k]^T @ E + z_k
with E = exp(aS - C) shared across classes. All masking is folded into the
stationary (lhsT) operands, so the device loop is pure matmul + Exp. Raw
numerators/denominators ship back in bf16; the divide+combine runs on host.

Sparsity: q rows where all mask_b are 0 contribute nothing on device (their
exp(-C) weight is in the host constants vks/zk), and p cols where all mask_a
are 0 output exactly 0 — both are compacted away on the host (~12% each for
random masks).

Sharding is 2D: 4 p-shards x 2 q-halves. With 8 cores all reading the same
fbn/ucomb the aggregate stream (~28MB) saturates HBM (~1.1TB/s observed) and
every core stalls; halving the q-range per core cuts the aggregate to ~17MB.
The q-partial numerators/denominators are summed on the HOST (free). Total
PE work is conserved.

The energy matmul runs in fp16 (full PE rate; fp32r also reaches 1 cyc/row
once the p-state ramp completes, but fp16 halves fbn DMA bytes): normalized
features are ~N(0, 1/256) so fp16 rounding adds ~1e-2 max logit noise.

44 warm-up matmuls bridge the head DMA wait so the PE p-state ramp completes
before the first real matmul. Matmul widths stay >= ~450 cols so the rhs
stream (~190ns) hides the ~116ns LDWEIGHTS issue floor. DMA streams are
few+large (issue costs ~700ns on the issuing engine), split across both
HWDGE queues in consumption order.
"""

import math
import numpy as np
import ml_dtypes

import concourse.bacc as bacc
import concourse.mybir as mybir
from concourse import tile
from concourse.bass_utils import run_bass_kernel_spmd

ALPHA = 100.0
# global logit shift: exp(alpha*S - CSHIFT) everywhere, with the "+1" weights
# of masked-out keys scaled by exp(-CSHIFT) on the host (vks/zk). Softmax is
# shift-invariant so this is exact; it keeps exp() in f32/bf16 range for
# logits up to CSHIFT+88 (observed max ~90).
CSHIFT = 60.0
N_CORES = 8
NP_SHARD = 4       # p-shards
NQ_SHARD = 2       # q-halves (cores 0-3: low half, 4-7: high half)
HW = 4096          # 64*64 spatial positions at feature resolution
UC = 195           # ucomb cols per q-tile: U0|U1|U2|mb0|mb1|mb2 = 64*3+3

F32 = mybir.dt.float32
F16 = mybir.dt.float16
BF16 = mybir.dt.bfloat16


def _interp_bilinear_ac(x, size):
    """torch F.interpolate bilinear align_corners=True; x: (C,H,W) float32."""
    x = np.ascontiguousarray(x, np.float32)
    H, W = x.shape[-2], x.shape[-1]
    h, w = size

    def coords(n_out, n_in):
        if n_out == 1:
            return np.zeros((1,), np.float32)
        return np.arange(n_out, dtype=np.float32) * np.float32((n_in - 1) / (n_out - 1))

    ry, rx = coords(h, H), coords(w, W)
    y0 = np.floor(ry).astype(np.int32)
    x0 = np.floor(rx).astype(np.int32)
    y1 = np.clip(y0 + 1, 0, H - 1)
    x1 = np.clip(x0 + 1, 0, W - 1)
    wy = (ry - y0.astype(np.float32))[None, :, None]
    wx = (rx - x0.astype(np.float32))[None, None, :]
    rows = x[:, y0, :] * (1.0 - wy) + x[:, y1, :] * wy
    return (rows[:, :, x0] * (1.0 - wx) + rows[:, :, x1] * wx).astype(np.float32)


_NC_CACHE = {}


def _build_program(NQH, PC):
    """NQH: q-tiles per core (half range); PC: p-cols per core (even)."""
    key = (NQH, PC)
    if key in _NC_CACHE:
        return _NC_CACHE[key]

    PH = PC // 2  # matmul width (one PSUM bank per [128, PH] f32 tile)
    nc = bacc.Bacc("TRN2", target_bir_lowering=False, debug=False,
                   num_devices=N_CORES)

    # fbn: q-tile-interleaved [c0(128)|c1(128)] per 128-q tile
    fbn_d = nc.dram_tensor("fbn", [128, NQH * 256], F16, kind="ExternalInput").ap()
    # fan: [c0(PC)|c1(PC)]
    fan_d = nc.dram_tensor("fan", [128, 2 * PC], F16, kind="ExternalInput").ap()
    uc_d = nc.dram_tensor("ucomb", [128, NQH * UC], BF16, kind="ExternalInput").ap()
    # out: [n1(PC) | n2(PC, rows 0:67)]
    out_d = nc.dram_tensor("out_nd", [128, 2 * PC], BF16,
                           kind="ExternalOutput").ap()

    EXP = mybir.ActivationFunctionType.Exp
    COPY = mybir.ActivationFunctionType.Copy
    NWARM = 44

    FCH = sorted(set([0, min(3, NQH), min(6, NQH), min(10, NQH), NQH]))
    UCH = sorted(set([0, min(2, NQH), min(8, NQH), NQH]))

    with tile.TileContext(nc) as tc:
        with (
            tc.tile_pool(name="io", bufs=1) as io,
            tc.tile_pool(name="big", bufs=1) as big,
            tc.tile_pool(name="expp", bufs=4) as expp,
            tc.tile_pool(name="spsum", bufs=4, space="PSUM") as spsum,
            tc.tile_pool(name="npsum", bufs=1, space="PSUM") as npsum,
            tc.tile_pool(name="fin", bufs=1) as fin,
        ):
            # one PSUM tile per p-half so no matmul output crosses a bank
            # boundary ([128, PH] f32 fits one 2KB bank for PH <= 512)
            n1h = [npsum.tile([128, PH], F32, tag=f"n1{h}", name=f"n1h{h}")
                   for h in range(2)]
            n2h = [npsum.tile([67, PH], F32, tag=f"n2{h}", name=f"n2h{h}")
                   for h in range(2)]

            # PE warm-up: full-array dummy matmuls with no DMA deps keep the
            # PE busy through the head DMA wait so the p-state ramp completes
            # before the first real matmul. They scribble into n1h[0], which
            # the first real accumulation resets (start=True).
            wz_sb = io.tile([128, 128], BF16, tag="wz")
            nc.vector.memset(wz_sb[:], 0.0)
            for _ in range(NWARM):
                nc.tensor.matmul(n1h[0][:, 0:128], wz_sb[:], wz_sb[:],
                                 start=True, stop=True)

            cb_sb = io.tile([128, 1], F32, tag="cb")
            nc.vector.memset(cb_sb[:], -CSHIFT)

            # fan halves (c0 gates the first matmul) on separate queues
            fan_sb = io.tile([128, 2 * PC], F16, tag="fan", name="fan_sb")
            nc.sync.dma_start(fan_sb[:, 0:PC], fan_d[:, 0:PC])
            nc.scalar.dma_start(fan_sb[:, PC:2 * PC], fan_d[:, PC:2 * PC])

            fbn_sb = big.tile([128, NQH * 256], F16, tag="fbn", name="fbn_sb")
            uc_sb = big.tile([128, NQH * UC], BF16, tag="ucomb")

            def fch(i):
                fs = slice(FCH[i] * 256, FCH[i + 1] * 256)
                return fbn_sb[:, fs], fbn_d[:, fs], FCH[i]

            def uch(i):
                us = slice(UCH[i] * UC, UCH[i + 1] * UC)
                return uc_sb[:, us], uc_d[:, us], UCH[i]

            nf, nu = len(FCH) - 1, len(UCH) - 1
            sp = [fch(i) for i in (0, 2) if i < nf] + \
                 [uch(i) for i in (1,) if i < nu]
            act = [fch(i) for i in (1, 3) if i < nf] + \
                  [uch(i) for i in (0, 2) if i < nu]
            for dst, src, _ in sorted(sp, key=lambda x: x[2]):
                nc.sync.dma_start(dst, src)
            for dst, src, _ in sorted(act, key=lambda x: x[2]):
                nc.scalar.dma_start(dst, src)

            o_sb = fin.tile([128, 2 * PC], BF16, tag="o")
            for t in range(NQH):
                b = t * 256
                u0 = t * UC
                first, last = (t == 0), (t == NQH - 1)
                for h in range(2):
                    pl, ph = h * PH, (h + 1) * PH
                    s_ps = spsum.tile([128, PH], F32, tag="s")
                    e_sb = expp.tile([128, PH], BF16, tag="e")
                    nc.tensor.matmul(s_ps[:], fbn_sb[:, b:b + 128],
                                     fan_sb[:, pl:ph], start=True, stop=False)
                    nc.tensor.matmul(s_ps[:], fbn_sb[:, b + 128:b + 256],
                                     fan_sb[:, PC + pl:PC + ph],
                                     start=False, stop=True)
                    nc.scalar.activation(e_sb[:], s_ps[:], EXP, scale=ALPHA,
                                         bias=cb_sb[:])
                    nc.tensor.matmul(n1h[h][:], uc_sb[:, u0:u0 + 128],
                                     e_sb[:], start=first, stop=last)
                    nc.tensor.matmul(n2h[h][:],
                                     uc_sb[:, u0 + 128:u0 + UC],
                                     e_sb[:], start=first, stop=last)
            # ship: PSUM->SBUF copies on two engines in parallel; the two
            # output DMAs split across both queues (n2 is only 67 rows) so
            # the tail transfer halves.
            for h in range(2):
                pl = h * PH
                nc.vector.tensor_copy(o_sb[:, pl:pl + PH], n1h[h][:])
                nc.scalar.activation(o_sb[0:67, PC + pl:PC + pl + PH],
                                     n2h[h][:], COPY)
            # split across both queues; full 128 rows each — a 67-row DMA
            # lands on a single engine (~17GB/s).
            nc.sync.dma_start(out_d[:, 0:PC], o_sb[:, 0:PC])
            nc.scalar.dma_start(out_d[:, PC:2 * PC], o_sb[:, PC:2 * PC])

    nc.compile()
    _NC_CACHE[key] = nc
    return nc


def _prep_inputs(unalign_fb, fa, fa_parse, fb, fb_parse):
    c2 = unalign_fb.shape[1]
    c = fa.shape[1]
    mask_a = (fa_parse[0, 1:, ::4, ::4].reshape(3, HW) != 0).astype(np.float32)
    mask_b = (fb_parse[0, 1:, ::4, ::4].reshape(3, HW) != 0).astype(np.float32)
    ufb = _interp_bilinear_ac(unalign_fb[0], (64, 64)).reshape(c2, HW)

    faf = np.ascontiguousarray(fa[0].reshape(c, HW), np.float32)
    fbf = np.ascontiguousarray(fb[0].reshape(c, HW), np.float32)
    faf = faf - faf.mean(axis=1, keepdims=True, dtype=np.float32)
    fbf = fbf - fbf.mean(axis=1, keepdims=True, dtype=np.float32)
    fan = faf / np.linalg.norm(faf, axis=0, keepdims=True)
    fbn = fbf / np.linalg.norm(fbf, axis=0, keepdims=True)

    # compaction: drop q rows with no mask_b and p cols with no mask_a
    qk = np.flatnonzero(mask_b.any(axis=0))
    pk = np.flatnonzero(mask_a.any(axis=0))
    NQ = max(1, math.ceil(len(qk) / 128))
    NQH = max(1, math.ceil(NQ / NQ_SHARD))      # q-tiles per core
    PC = max(8, math.ceil(len(pk) / (NP_SHARD * 8)) * 8)  # p-cols per core
    qpad, ppad = NQH * NQ_SHARD * 128, PC * NP_SHARD

    fbnk = np.zeros((c, qpad), np.float32)
    fbnk[:, :len(qk)] = fbn[:, qk]
    fank = np.zeros((c, ppad), np.float32)
    fank[:, :len(pk)] = fan[:, pk]
    mbk = np.zeros((3, qpad), np.float32)
    mbk[:, :len(qk)] = mask_b[:, qk]
    ufbk = np.zeros((c2, qpad), np.float32)
    ufbk[:, :len(qk)] = ufb[:, qk]

    # stationary operands for the numerator/denominator matmuls, tiled per
    # 128-q block: [U0|U1|U2|mb0|mb1|mb2] transposed to [q,cols]
    NQT = qpad // 128
    U = ufbk[None] * mbk[:, None, :]                       # (3,64,qpad)
    ucomb = np.empty((128, NQT * UC), np.float32)
    Ut = U.transpose(2, 0, 1).reshape(qpad, 3 * 64)        # (qpad,192) q-major
    mbt = mbk.T
    for t in range(NQT):
        qs = slice(t * 128, (t + 1) * 128)
        ucomb[:, t * UC:t * UC + 192] = Ut[qs]
        ucomb[:, t * UC + 192:t * UC + 195] = mbt[qs]
    ucomb = ucomb.astype(ml_dtypes.bfloat16)

    # fbn q-tile-interleaved: [128, NQT*256], tile t = [c0 cols | c1 cols]
    fbn16 = np.ascontiguousarray(
        fbnk.reshape(2, 128, NQT, 128).transpose(1, 2, 0, 3).reshape(128, NQT * 256)
    ).astype(np.float16)
    fan3 = fank.reshape(2, 128, ppad)
    in_maps = []
    for i in range(N_CORES):
        pi = i % NP_SHARD
        qi = i // NP_SHARD
        ps = slice(pi * PC, (pi + 1) * PC)
        fan16 = np.ascontiguousarray(
            np.concatenate([fan3[0][:, ps], fan3[1][:, ps]], axis=1)
        ).astype(np.float16)
        ts = slice(qi * NQH * 256, (qi + 1) * NQH * 256)
        us = slice(qi * NQH * UC, (qi + 1) * NQH * UC)
        in_maps.append({
            "fbn": np.ascontiguousarray(fbn16[:, ts]),
            "fan": fan16,
            "ucomb": np.ascontiguousarray(ucomb[:, us]),
        })

    # host-epilogue constants (computed from the FULL masks, so dropped q
    # rows' exp(-C) contributions stay exact)
    esc = np.float32(np.exp(-CSHIFT))
    normk = np.maximum(mask_a[:, pk].sum(axis=0), 1.0)
    gak = (mask_a[:, pk] / normk[None, :]).astype(np.float32)   # (3,len(pk))
    vks = (ufb @ (1.0 - mask_b).T).astype(np.float32) * esc     # (64,3)
    zk = ((1.0 - mask_b).sum(axis=1).astype(np.float32) * esc)  # (3,)
    return in_maps, (NQH, PC, pk, gak, vks, zk)


def _run(inputs, trace=False, trace_cores=None):
    unalign_fb = np.asarray(inputs["unalign_fb"], np.float32)
    fa = np.asarray(inputs["fa"], np.float32)
    fa_parse = np.asarray(inputs["fa_parse"])
    fb = np.asarray(inputs["fb"], np.float32)
    fb_parse = np.asarray(inputs["fb_parse"])

    in_maps, (NQH, PC, pk, gak, vks, zk) = _prep_inputs(
        unalign_fb, fa, fa_parse, fb, fb_parse)
    nc = _build_program(NQH, PC)
    res = run_bass_kernel_spmd(nc, in_maps, core_ids=list(range(N_CORES)),
                               trace=trace, trace_cores=trace_cores)

    c2 = unalign_fb.shape[1]
    npk = len(pk)
    aligned_k = np.zeros((c2, NP_SHARD * PC), np.float32)
    for pi in range(NP_SHARD):
        # sum the two q-half partials on the host (exact in f32)
        nd = np.zeros((195, PC), np.float32)
        for qi in range(NQ_SHARD):
            o = np.asarray(res.results[qi * NP_SHARD + pi]["out_nd"],
                           np.float32)
            nd += np.concatenate([o[:, 0:PC], o[0:67, PC:2 * PC]])
        ps = slice(pi * PC, (pi + 1) * PC)
        pglob = np.arange(pi * PC, (pi + 1) * PC)
        valid = pglob < npk
        ga_s = np.zeros((3, PC), np.float32)
        ga_s[:, valid] = gak[:, pglob[valid]]
        for k in range(3):
            numer = nd[64 * k:64 * k + 64] + vks[:, k:k + 1]
            denom = nd[192 + k] + zk[k]
            aligned_k[:, ps] += (ga_s[k] / denom)[None, :] * numer
    aligned = np.zeros((c2, HW), np.float32)
    aligned[:, pk] = aligned_k[:, :npk]
    out = _interp_bilinear_ac(aligned.reshape(c2, 64, 64), (256, 256))
    return out[None], res


def kernel(**inputs):
    out, _ = _run(inputs)
    return out


# revision 32
# speedup vs baseline: 1.0176x; 1.0176x over previous
"""Trainium2 Bass kernel for LocalSemanticAlignment (sparse_attention).

Pipeline (reference semantics):
  masks   = parse[:,1:] downsampled 256->64 (nearest, stride-4)
  ufb     = bilinear-AC downsample of unalign_fb to 64x64        (host)
  fan/fbn = per-channel-centered, per-column L2-normalized fa/fb (host)
  S[q,p]  = fbn^T fan                                            (device, fp16 matmul)
  per class k: w_k = where(mask_b[q], exp(alpha*S - C), exp(-C)) masked
  softmax over q (shift C is exact; see CSHIFT)
  warped_k = ufb @ softmax  ->  combined over k with mask_a / counts
  output  = bilinear-AC upsample of aligned to 256x256           (host)

Key identity used on device: w_k = mask_b[q]*exp(aS-C) + (1-mask_b[q])e^-C, so
  numer_k = (ufb*mask_b[k])^T @ E + const_k,  denom_k = mask_b[k]^T @ E + z_k
with E = exp(aS - C) shared across classes. All masking is folded into the
stationary (lhsT) operands, so the device loop is pure matmul + Exp. Raw
numerators/denominators ship back in bf16; the divide+combine runs on host.

Sparsity: q rows where all mask_b are 0 contribute nothing on device (their
exp(-C) weight is in the host constants vks/zk), and p cols where all mask_a
are 0 output exactly 0 — both are compacted away on the host (~12% each for
random masks).

Sharding is 2D: 4 p-shards x 2 q-halves. With 8 cores all reading the same
fbn/ucomb the aggregate stream (~28MB) saturates HBM (~1.1TB/s observed) and
every core stalls; halving the q-range per core cuts the aggregate to ~17MB.
The q-partial numerators/denominators are summed on the HOST (free). Total
PE work is conserved.

The energy matmul runs in fp16 (full PE rate; fp32r also reaches 1 cyc/row
once the p-state ramp completes, but fp16 halves fbn DMA bytes): normalized
features are ~N(0, 1/256) so fp16 rounding adds ~1e-2 max logit noise.

44 warm-up matmuls bridge the head DMA wait so the PE p-state ramp completes
before the first real matmul. Matmul widths stay >= ~450 cols so the rhs
stream (~190ns) hides the ~116ns LDWEIGHTS issue floor. DMA streams are
few+large (issue costs ~700ns on the issuing engine), split across both
HWDGE queues in consumption order.
"""

import math
import numpy as np
import ml_dtypes

import concourse.bacc as bacc
import concourse.mybir as mybir
from concourse import tile
from concourse.bass_utils import run_bass_kernel_spmd

ALPHA = 100.0
# global logit shift: exp(alpha*S - CSHIFT) everywhere, with the "+1" weights
# of masked-out keys scaled by exp(-CSHIFT) on the host (vks/zk). Softmax is
# shift-invariant so this is exact; it keeps exp() in f32/bf16 range for
# logits up to CSHIFT+88 (observed max ~90).
CSHIFT = 60.0
N_CORES = 8
NP_SHARD = 4       # p-shards
NQ_SHARD = 2       # q-halves (cores 0-3: low half, 4-7: high half)
HW = 4096          # 64*64 spatial positions at feature resolution
UC = 195           # ucomb cols per q-tile: U0|U1|U2|mb0|mb1|mb2 = 64*3+3

F32 = mybir.dt.float32
F16 = mybir.dt.float16
BF16 = mybir.dt.bfloat16


def _interp_bilinear_ac(x, size):
    """torch F.interpolate bilinear align_corners=True; x: (C,H,W) float32."""
    x = np.ascontiguousarray(x, np.float32)
    H, W = x.shape[-2], x.shape[-1]
    h, w = size

    def coords(n_out, n_in):
        if n_out == 1:
            return np.zeros((1,), np.float32)
        return np.arange(n_out, dtype=np.float32) * np.float32((n_in - 1) / (n_out - 1))

    ry, rx = coords(h, H), coords(w, W)
    y0 = np.floor(ry).astype(np.int32)
    x0 = np.floor(rx).astype(np.int32)
    y1 = np.clip(y0 + 1, 0, H - 1)
    x1 = np.clip(x0 + 1, 0, W - 1)
    wy = (ry - y0.astype(np.float32))[None, :, None]
    wx = (rx - x0.astype(np.float32))[None, None, :]
    rows = x[:, y0, :] * (1.0 - wy) + x[:, y1, :] * wy
    return (rows[:, :, x0] * (1.0 - wx) + rows[:, :, x1] * wx).astype(np.float32)


_NC_CACHE = {}


def _build_program(NQH, PC):
    """NQH: q-tiles per core (half range); PC: p-cols per core (even)."""
    key = (NQH, PC)
    if key in _NC_CACHE:
        return _NC_CACHE[key]

    PH = PC // 2  # matmul width (one PSUM bank per [128, PH] f32 tile)
    nc = bacc.Bacc("TRN2", target_bir_lowering=False, debug=False,
                   num_devices=N_CORES)

    # fbn: q-tile-interleaved [c0(128)|c1(128)] per 128-q tile
    fbn_d = nc.dram_tensor("fbn", [128, NQH * 256], F16, kind="ExternalInput").ap()
    # fan: pass-major quarters [A:c0(PH)|A:c1(PH)|B:c0(PH)|B:c1(PH)]
    fan_d = nc.dram_tensor("fan", [128, 2 * PC], F16, kind="ExternalInput").ap()
    uc_d = nc.dram_tensor("ucomb", [128, NQH * UC], BF16, kind="ExternalInput").ap()
    # out: [n1(PC) | n2(PC, rows 0:67)]
    out_d = nc.dram_tensor("out_nd", [128, 2 * PC], BF16,
                           kind="ExternalOutput").ap()

    EXP = mybir.ActivationFunctionType.Exp
    COPY = mybir.ActivationFunctionType.Copy
    NWARM = 44

    FCH = sorted(set([0, min(2, NQH), min(5, NQH), min(9, NQH), NQH]))
    UCH = sorted(set([0, min(2, NQH), min(5, NQH), min(9, NQH), NQH]))

    with tile.TileContext(nc) as tc:
        with (
            tc.tile_pool(name="io", bufs=1) as io,
            tc.tile_pool(name="big", bufs=1) as big,
            tc.tile_pool(name="expp", bufs=4) as expp,
            tc.tile_pool(name="spsum", bufs=4, space="PSUM") as spsum,
            tc.tile_pool(name="npsum", bufs=1, space="PSUM") as npsum,
            tc.tile_pool(name="fin", bufs=1) as fin,
        ):
            # one PSUM tile per p-half so no matmul output crosses a bank
            # boundary ([128, PH] f32 fits one 2KB bank for PH <= 512)
            n1h = [npsum.tile([128, PH], F32, tag=f"n1{h}", name=f"n1h{h}")
                   for h in range(2)]
            n2h = [npsum.tile([67, PH], F32, tag=f"n2{h}", name=f"n2h{h}")
                   for h in range(2)]

            # PE warm-up: full-array dummy matmuls with no DMA deps keep the
            # PE busy through the head DMA wait so the p-state ramp completes
            # before the first real matmul. They scribble into n1h[0], which
            # the first real accumulation resets (start=True).
            wz_sb = io.tile([128, 128], BF16, tag="wz")
            nc.vector.memset(wz_sb[:], 0.0)
            for _ in range(NWARM):
                nc.tensor.matmul(n1h[0][:, 0:128], wz_sb[:], wz_sb[:],
                                 start=True, stop=True)

            cb_sb = io.tile([128, 1], F32, tag="cb")
            nc.vector.memset(cb_sb[:], -CSHIFT)

            # pass-A fan quarters split across both queues (113KB each
            # gates the first matmul); pass-B fan arrives mid-stream
            fan_sb = io.tile([128, 2 * PC], F16, tag="fan", name="fan_sb")
            nc.sync.dma_start(fan_sb[:, 0:PH], fan_d[:, 0:PH])
            nc.scalar.dma_start(fan_sb[:, PH:2 * PH], fan_d[:, PH:2 * PH])

            fbn_sb = big.tile([128, NQH * 256], F16, tag="fbn", name="fbn_sb")
            uc_sb = big.tile([128, NQH * UC], BF16, tag="ucomb")

            def fch(i):
                fs = slice(FCH[i] * 256, FCH[i + 1] * 256)
                return fbn_sb[:, fs], fbn_d[:, fs], FCH[i]

            def uch(i):
                us = slice(UCH[i] * UC, UCH[i + 1] * UC)
                return uc_sb[:, us], uc_d[:, us], UCH[i]

            nf, nu = len(FCH) - 1, len(UCH) - 1
            sp = [fch(i) for i in (0, 1, 3) if i < nf] + \
                 [uch(i) for i in (2,) if i < nu]
            act = [fch(i) for i in (2,) if i < nf] + \
                  [uch(i) for i in (0, 1, 3) if i < nu]
            for dst, src, _ in sorted(sp, key=lambda x: x[2]):
                nc.sync.dma_start(dst, src)
            # pass-B fan is needed only ~11us after first S
            nc.sync.dma_start(fan_sb[:, 2 * PH:4 * PH], fan_d[:, 2 * PH:4 * PH])
            for dst, src, _ in sorted(act, key=lambda x: x[2]):
                nc.scalar.dma_start(dst, src)

            # two passes over all q-tiles, one per p-half: pass A's output
            # ships mid-kernel (fully overlapped with pass B), so only half
            # the output transfer sits in the tail; fbn/ucomb are resident
            # after pass A so pass B needs no new input DMA.
            o_sb = fin.tile([128, 2 * PC], BF16, tag="o")
            for h in range(2):
                fq = 2 * PH * h      # fan quarter base for this pass
                for t in range(NQH):
                    b = t * 256
                    u0 = t * UC
                    first, last = (t == 0), (t == NQH - 1)
                    s_ps = spsum.tile([128, PH], F32, tag="s")
                    e_sb = expp.tile([128, PH], BF16, tag="e")
                    nc.tensor.matmul(s_ps[:], fbn_sb[:, b:b + 128],
                                     fan_sb[:, fq:fq + PH],
                                     start=True, stop=False)
                    nc.tensor.matmul(s_ps[:], fbn_sb[:, b + 128:b + 256],
                                     fan_sb[:, fq + PH:fq + 2 * PH],
                                     start=False, stop=True)
                    nc.scalar.activation(e_sb[:], s_ps[:], EXP, scale=ALPHA,
                                         bias=cb_sb[:])
                    nc.tensor.matmul(n1h[h][:], uc_sb[:, u0:u0 + 128],
                                     e_sb[:], start=first, stop=last)
                    nc.tensor.matmul(n2h[h][:],
                                     uc_sb[:, u0 + 128:u0 + UC],
                                     e_sb[:], start=first, stop=last)
                # ship this half: PSUM->SBUF copies on two engines in
                # parallel; out DMAs split across both queues, full 128 rows
                # each (a 67-row DMA lands on a single engine at ~17GB/s).
                pl = h * PH
                nc.vector.tensor_copy(o_sb[:, pl:pl + PH], n1h[h][:])
                nc.scalar.activation(o_sb[0:67, PC + pl:PC + pl + PH],
                                     n2h[h][:], COPY)
                nc.sync.dma_start(out_d[:, pl:pl + PH], o_sb[:, pl:pl + PH])
                nc.scalar.dma_start(out_d[:, PC + pl:PC + pl + PH],
                                    o_sb[:, PC + pl:PC + pl + PH])

    nc.compile()
    _NC_CACHE[key] = nc
    return nc


def _prep_inputs(unalign_fb, fa, fa_parse, fb, fb_parse):
    c2 = unalign_fb.shape[1]
    c = fa.shape[1]
    mask_a = (fa_parse[0, 1:, ::4, ::4].reshape(3, HW) != 0).astype(np.float32)
    mask_b = (fb_parse[0, 1:, ::4, ::4].reshape(3, HW) != 0).astype(np.float32)
    ufb = _interp_bilinear_ac(unalign_fb[0], (64, 64)).reshape(c2, HW)

    faf = np.ascontiguousarray(fa[0].reshape(c, HW), np.float32)
    fbf = np.ascontiguousarray(fb[0].reshape(c, HW), np.float32)
    faf = faf - faf.mean(axis=1, keepdims=True, dtype=np.float32)
    fbf = fbf - fbf.mean(axis=1, keepdims=True, dtype=np.float32)
    fan = faf / np.linalg.norm(faf, axis=0, keepdims=True)
    fbn = fbf / np.linalg.norm(fbf, axis=0, keepdims=True)

    # compaction: drop q rows with no mask_b and p cols with no mask_a
    qk = np.flatnonzero(mask_b.any(axis=0))
    pk = np.flatnonzero(mask_a.any(axis=0))
    NQ = max(1, math.ceil(len(qk) / 128))
    NQH = max(1, math.ceil(NQ / NQ_SHARD))      # q-tiles per core
    PC = max(8, math.ceil(len(pk) / (NP_SHARD * 8)) * 8)  # p-cols per core
    qpad, ppad = NQH * NQ_SHARD * 128, PC * NP_SHARD

    fbnk = np.zeros((c, qpad), np.float32)
    fbnk[:, :len(qk)] = fbn[:, qk]
    fank = np.zeros((c, ppad), np.float32)
    fank[:, :len(pk)] = fan[:, pk]
    mbk = np.zeros((3, qpad), np.float32)
    mbk[:, :len(qk)] = mask_b[:, qk]
    ufbk = np.zeros((c2, qpad), np.float32)
    ufbk[:, :len(qk)] = ufb[:, qk]

    # stationary operands for the numerator/denominator matmuls, tiled per
    # 128-q block: [U0|U1|U2|mb0|mb1|mb2] transposed to [q,cols]
    NQT = qpad // 128
    U = ufbk[None] * mbk[:, None, :]                       # (3,64,qpad)
    ucomb = np.empty((128, NQT * UC), np.float32)
    Ut = U.transpose(2, 0, 1).reshape(qpad, 3 * 64)        # (qpad,192) q-major
    mbt = mbk.T
    for t in range(NQT):
        qs = slice(t * 128, (t + 1) * 128)
        ucomb[:, t * UC:t * UC + 192] = Ut[qs]
        ucomb[:, t * UC + 192:t * UC + 195] = mbt[qs]
    ucomb = ucomb.astype(ml_dtypes.bfloat16)

    # fbn q-tile-interleaved: [128, NQT*256], tile t = [c0 cols | c1 cols]
    fbn16 = np.ascontiguousarray(
        fbnk.reshape(2, 128, NQT, 128).transpose(1, 2, 0, 3).reshape(128, NQT * 256)
    ).astype(np.float16)
    fan3 = fank.reshape(2, 128, ppad)
    in_maps = []
    for i in range(N_CORES):
        pi = i % NP_SHARD
        qi = i // NP_SHARD
        ps = slice(pi * PC, (pi + 1) * PC)
        PH_ = PC // 2
        c0, c1 = fan3[0][:, ps], fan3[1][:, ps]
        fan16 = np.ascontiguousarray(np.concatenate(
            [c0[:, 0:PH_], c1[:, 0:PH_], c0[:, PH_:PC], c1[:, PH_:PC]],
            axis=1)).astype(np.float16)
        ts = slice(qi * NQH * 256, (qi + 1) * NQH * 256)
        us = slice(qi * NQH * UC, (qi + 1) * NQH * UC)
        in_maps.append({
            "fbn": np.ascontiguousarray(fbn16[:, ts]),
            "fan": fan16,
            "ucomb": np.ascontiguousarray(ucomb[:, us]),
        })

    # host-epilogue constants (computed from the FULL masks, so dropped q
    # rows' exp(-C) contributions stay exact)
    esc = np.float32(np.exp(-CSHIFT))
    normk = np.maximum(mask_a[:, pk].sum(axis=0), 1.0)
    gak = (mask_a[:, pk] / normk[None, :]).astype(np.float32)   # (3,len(pk))
    vks = (ufb @ (1.0 - mask_b).T).astype(np.float32) * esc     # (64,3)
    zk = ((1.0 - mask_b).sum(axis=1).astype(np.float32) * esc)  # (3,)
    return in_maps, (NQH, PC, pk, gak, vks, zk)


def _run(inputs, trace=False, trace_cores=None):
    unalign_fb = np.asarray(inputs["unalign_fb"], np.float32)
    fa = np.asarray(inputs["fa"], np.float32)
    fa_parse = np.asarray(inputs["fa_parse"])
    fb = np.asarray(inputs["fb"], np.float32)
    fb_parse = np.asarray(inputs["fb_parse"])

    in_maps, (NQH, PC, pk, gak, vks, zk) = _prep_inputs(
        unalign_fb, fa, fa_parse, fb, fb_parse)
    nc = _build_program(NQH, PC)
    res = run_bass_kernel_spmd(nc, in_maps, core_ids=list(range(N_CORES)),
                               trace=trace, trace_cores=trace_cores)

    c2 = unalign_fb.shape[1]
    npk = len(pk)
    aligned_k = np.zeros((c2, NP_SHARD * PC), np.float32)
    for pi in range(NP_SHARD):
        # sum the two q-half partials on the host (exact in f32)
        nd = np.zeros((195, PC), np.float32)
        for qi in range(NQ_SHARD):
            o = np.asarray(res.results[qi * NP_SHARD + pi]["out_nd"],
                           np.float32)
            nd += np.concatenate([o[:, 0:PC], o[0:67, PC:2 * PC]])
        ps = slice(pi * PC, (pi + 1) * PC)
        pglob = np.arange(pi * PC, (pi + 1) * PC)
        valid = pglob < npk
        ga_s = np.zeros((3, PC), np.float32)
        ga_s[:, valid] = gak[:, pglob[valid]]
        for k in range(3):
            numer = nd[64 * k:64 * k + 64] + vks[:, k:k + 1]
            denom = nd[192 + k] + zk[k]
            aligned_k[:, ps] += (ga_s[k] / denom)[None, :] * numer
    aligned = np.zeros((c2, HW), np.float32)
    aligned[:, pk] = aligned_k[:, :npk]
    out = _interp_bilinear_ac(aligned.reshape(c2, 64, 64), (256, 256))
    return out[None], res


def kernel(**inputs):
    out, _ = _run(inputs)
    return out
